# revision 1
# baseline (speedup 1.0000x reference)
"""Trainium2 Bass kernel for DenseLanguageGuidanceModule.

Math (per batch b):
    fk_l = fl @ W_lk + b_lk            [77, 512]
    fv-side projections are folded away algebraically:
      a_raw = (fk_l @ W_vk^T) @ fv^T + (fk_l @ b_vk) 1^T   (/= sqrt(512))
      fa_v  = diag(1/s1) (E @ fv) @ W_vv + b_vv,  E = exp(a_raw/sqrt(512))
      fm    = diag(1/s2) E^T @ (fv_l @ fa_v^T)
      out   = fm @ W_m + b_m
    where s1 = row sums of E, s2 = column sums of E.

Distribution: pure data-parallel over batch B=32 across 8 NeuronCores
(4 batches per core), weights replicated. No collectives.

All matmuls run in float32r (TF32-like: 11 mantissa bits, fp32 accumulate)
which is full PE speed for free-dim >= 256. Inputs are pre-rounded to f32r
on the host (RNE, keep top 20 bits) so on-device rounding is a no-op.
"""
import sys

sys.path.insert(0, "/opt/trn_rl_repo")

import numpy as np

import concourse.bass as bass  # noqa: E402
import concourse.tile as tile  # noqa: E402
from concourse import bacc, mybir  # noqa: E402
from concourse.bass_utils import run_bass_kernel_spmd  # noqa: E402

P = 128
NCORES = 8
B = 32
BL = B // NCORES          # 4 batches per core
NV, DV = 1024, 768        # vision tokens / dim
NL, DL = 77, 512          # language tokens / dim
D = 512                   # shared feature dim
OD = 768                  # output dim
NLB = NL * BL             # 308: l-dim stacked across local batches

F32R = mybir.dt.float32r
F32 = mybir.dt.float32
ISQD = 1.0 / float(np.sqrt(np.float32(D)))

AF = mybir.ActivationFunctionType


def round_f32r(x: np.ndarray) -> np.ndarray:
    """RNE-round fp32 to f32r (1s+8e+11m in the top 20 bits)."""
    u = np.ascontiguousarray(x, dtype=np.float32).view(np.uint32)
    low = u & np.uint32(0xFFF)
    base = u & np.uint32(0xFFFFF000)
    lsb = (u >> np.uint32(12)) & np.uint32(1)
    up = (low > 0x800) | ((low == 0x800) & (lsb == 1))
    return (base + np.where(up, np.uint32(0x1000), np.uint32(0))).view(np.float32)


def _build():
    nc = bacc.Bacc("TRN2", target_bir_lowering=False)

    fv_d = nc.dram_tensor("fv", [BL, NV, DV], F32R, kind="ExternalInput")
    fl_d = nc.dram_tensor("fl", [BL, NL, DL], F32R, kind="ExternalInput")
    wkc_d = nc.dram_tensor("wkc", [DL, DV], F32R, kind="ExternalInput")
    wvc_d = nc.dram_tensor("wvc", [DL, DV], F32R, kind="ExternalInput")
    wm_d = nc.dram_tensor("wm", [97, OD], F32R, kind="ExternalInput")
    wc_d = nc.dram_tensor("wc", [DL], F32R, kind="ExternalInput")
    c2_d = nc.dram_tensor("c2", [DV], F32, kind="ExternalInput")
    cc_d = nc.dram_tensor("cc", [1], F32, kind="ExternalInput")
    c2v_d = nc.dram_tensor("c2v", [DV], F32, kind="ExternalInput")
    wcv_d = nc.dram_tensor("wcv", [DL], F32R, kind="ExternalInput")
    ccv_d = nc.dram_tensor("ccv", [1], F32, kind="ExternalInput")
    iden_d = nc.dram_tensor("iden", [P, P], F32R, kind="ExternalInput")
    onesr_d = nc.dram_tensor("onesr", [1, 512], F32R, kind="ExternalInput")
    onesc_d = nc.dram_tensor("onesc", [P, 1], F32R, kind="ExternalInput")
    zeros_d = nc.dram_tensor("zeros", [19, NV], F32R, kind="ExternalInput")
    out_d = nc.dram_tensor("out", [BL, NV, OD], F32, kind="ExternalOutput")

    with tile.TileContext(nc) as tc:
        with (
            tc.tile_pool(name="consts", bufs=1) as cp,
            tc.tile_pool(name="lph", bufs=1) as lp,
            tc.tile_pool(name="fvn", bufs=2) as fvnp,
            tc.tile_pool(name="fvt", bufs=3) as fvtp,
            tc.tile_pool(name="eb", bufs=2) as ebp,
            tc.tile_pool(name="sm", bufs=2) as smp,
            tc.tile_pool(name="outp", bufs=4) as outp,
            tc.tile_pool(name="tp", bufs=4, space="PSUM") as tp,       # 1-bank slots
            tc.tile_pool(name="acc", bufs=2, space="PSUM") as accp,    # 2-bank slots
        ):
            # ---------------- constants (early: identity only) ----------------
            iden = cp.tile([P, P], F32R)
            nc.sync.dma_start(iden, iden_d[:, :])
            # ---------------- language phase (batched over BL) ----------------
            lph_tmp = tc.tile_pool(name="lphtmp", bufs=1)
            lpt = lph_tmp.__enter__()
            # FLT = fl_all^T  [512(D1 on p), 308]
            FLT = lpt.tile([P, 4, NLB], F32R)
            with tc.tile_pool(name="fln", bufs=1) as flnp:
                fl_flat = fl_d.rearrange("b l d -> (b l) d")
                row_tiles = [(0, P), (P, P), (2 * P, NLB - 2 * P)]
                for half in range(2):
                    c0 = half * 256
                    FLn = flnp.tile([P, 3, 256], F32R, tag="fln")
                    for i, (r0, sz) in enumerate(row_tiles):
                        nc.sync.dma_start(
                            FLn[:sz, i, :], fl_flat[r0 : r0 + sz, c0 : c0 + 256]
                        )
                    for fb in range(2):
                        ps = tp.tile([P, 384], F32R, tag="tp")
                        for i in range(3):
                            nc.tensor.transpose(
                                ps[:, i * P : (i + 1) * P],
                                FLn[:, i, fb * P : (fb + 1) * P],
                                iden,
                            )
                        fbo = half * 2 + fb
                        nc.vector.tensor_copy(FLT[:, fbo, : 2 * P], ps[:, : 2 * P])
                        nc.vector.tensor_copy(
                            FLT[:, fbo, 2 * P :], ps[:, 2 * P : 2 * P + (NLB - 2 * P)]
                        )

            # ---------------- remaining constants, interleaved with fv[0] ----------------
            FVn0 = fvnp.tile([P, 8, DV], F32R, tag="fvn")
            fvb0 = fv_d[0].rearrange("(t p) d -> p t d", p=P)
            Wkc = lpt.tile([P, 4, DV], F32R)
            nc.sync.dma_start(Wkc, wkc_d.rearrange("(ko p) m -> p ko m", p=P))
            c2t = cp.tile([P, 6], F32)
            nc.sync.dma_start(c2t, c2_d.rearrange("(ko p) -> p ko", p=P))
            Wvc = lpt.tile([P, 4, DV], F32R)
            nc.sync.dma_start(Wvc, wvc_d.rearrange("(ko p) m -> p ko m", p=P))
            c2vt = cp.tile([P, 6], F32)
            nc.sync.dma_start(c2vt, c2v_d.rearrange("(ko p) -> p ko", p=P))
            wcvt = cp.tile([P, 4], F32R)
            nc.sync.dma_start(wcvt, wcv_d.rearrange("(ko p) -> p ko", p=P))
            ccvt = cp.tile([1, 1], F32)
            nc.sync.dma_start(ccvt, ccv_d[None, :])

            wct = cp.tile([P, 4], F32R)
            nc.sync.dma_start(wct, wc_d.rearrange("(ko p) -> p ko", p=P))
            cct = cp.tile([1, 1], F32)
            nc.sync.dma_start(cct, cc_d[None, :])
            for q in range(4):
                nc.sync.dma_start(FVn0[:, 2 * q : 2 * q + 2, :], fvb0[:, 2 * q : 2 * q + 2, :])
            onesr = cp.tile([1, 512], F32R)
            nc.sync.dma_start(onesr, onesr_d[:, :])
            onesc = cp.tile([P, 1], F32R)
            nc.sync.dma_start(onesc, onesc_d[:, :])


            # FWVT = (fv_l @ W_vv^T)^T = (W_lv@W_vv^T)^T @ fl^T + c2v : [768, 308]
            FWVT = lp.tile([P, 6, NLB], F32R)
            for mv in range(6):
                ps = tp.tile([P, NLB], F32, tag="tp")
                for ko in range(4):
                    nc.tensor.matmul(
                        ps, Wvc[:, ko, mv * P : (mv + 1) * P], FLT[:, ko, :],
                        start=(ko == 0), stop=(ko == 3),
                    )
                if mv % 2 == 0:
                    nc.vector.tensor_scalar_add(FWVT[:, mv, :], ps, c2vt[:, mv, None])
                else:
                    nc.scalar.activation(
                        FWVT[:, mv, :], ps, AF.Identity, bias=c2vt[:, mv, None]
                    )

            # Cv = (fv_l @ b_vv)^T : [1, 308]
            Cv = lp.tile([1, NLB], F32R)
            pscv = tp.tile([1, NLB], F32, tag="tp")
            for ko in range(4):
                nc.tensor.matmul(
                    pscv, wcvt[:, ko, None], FLT[:, ko, :],
                    start=(ko == 0), stop=(ko == 3),
                )
            nc.vector.tensor_scalar_add(Cv, pscv, ccvt[:, :])

            # GT = g^T = (W_lk @ W_vk^T)^T @ fl^T + c2 : [768, 308]
            GT = lp.tile([P, 6, NLB], F32R)
            for mv in range(6):
                ps = tp.tile([P, NLB], F32, tag="tp")
                for ko in range(4):
                    nc.tensor.matmul(
                        ps, Wkc[:, ko, mv * P : (mv + 1) * P], FLT[:, ko, :],
                        start=(ko == 0), stop=(ko == 3),
                    )
                if mv % 2 == 0:
                    nc.vector.tensor_scalar_add(GT[:, mv, :], ps, c2t[:, mv, None])
                else:
                    nc.scalar.activation(
                        GT[:, mv, :], ps, AF.Identity, bias=c2t[:, mv, None]
                    )

            # C = (fk_l @ b_vk)^T : [1, 308]
            Cst = lp.tile([1, NLB], F32R)
            psc = tp.tile([1, NLB], F32, tag="tp")
            for ko in range(4):
                nc.tensor.matmul(
                    psc, wct[:, ko, None], FLT[:, ko, :],
                    start=(ko == 0), stop=(ko == 3),
                )
            nc.vector.tensor_scalar_add(Cst, psc, cct[:, :])
            lph_tmp.__exit__(None, None, None)
            fvn3_pool = tc.tile_pool(name="fvn3", bufs=1)
            fvn3 = fvn3_pool.__enter__()

            # late-use constants (not needed until final)
            Wm = cp.tile([97, OD], F32R)
            nc.sync.dma_start(Wm, wm_d[:, :])


            # persistent FMT ping-pong pair; filler rows 77..95 zeroed once
            FMTs = []
            for _i in range(2):
                _f = lp.tile([97, NV], F32R, tag=f"FMT{_i}")
                nc.sync.dma_start(_f[NL:96, :], zeros_d[:, :])
                FMTs.append(_f)

            # ---------------- per-batch vision phase ----------------
            pending_finals = []
            vstate = {}

            def _emit_tg(FVn, tg):
                fvth = fvtp.tile([P, 6, 512], F32R, tag="fvt")
                for ko in range(6):
                    ps = tp.tile([P, 512], F32R, tag="tp")
                    for tt in range(4):
                        t = tg * 4 + tt
                        nc.tensor.transpose(
                            ps[:, tt * P : (tt + 1) * P],
                            FVn[:, t, ko * P : (ko + 1) * P],
                            iden,
                        )
                    if (ko + tg) % 2 == 0:
                        nc.vector.tensor_copy(fvth[:, ko, :], ps)
                    else:
                        nc.scalar.activation(fvth[:, ko, :], ps, AF.Copy)
                return fvth

            def _emit_load_tg0(nb):
                if nb == 0:
                    FVn = FVn0
                else:
                    pool_b = fvn3 if nb == 2 else fvnp
                    FVn = pool_b.tile([P, 8, DV], F32R, tag="fvn")
                    fvb = fv_d[nb].rearrange("(t p) d -> p t d", p=P)
                    for q in range(4):
                        nc.sync.dma_start(
                            FVn[:, 2 * q : 2 * q + 2, :], fvb[:, 2 * q : 2 * q + 2, :]
                        )
                vstate[nb] = [FVn, [_emit_tg(FVn, 0)], None]

            def _emit_araw_E(nb, FVn, FVTh):
                nls = nb * NL
                if len(FVTh) == 1:
                    FVTh.append(_emit_tg(FVn, 1))
                # a_raw = g @ fv^T + c 1^T  -> psum [77, 1024]
                psa = accp.tile([NL, NV], F32, tag="acc")
                for nv in range(2):
                    sl = psa[:, nv * 512 : (nv + 1) * 512]
                    for ko in range(6):
                        nc.tensor.matmul(
                            sl, GT[:, ko, nls : nls + NL],
                            FVTh[nv][:, ko, :],
                            start=(ko == 0), stop=False,
                        )
                    nc.tensor.matmul(
                        sl, Cst[:1, nls : nls + NL],
                        onesr[:1, :512],
                        start=False, stop=True,
                    )
                # E = exp(a/sqrt(D)); s1 = row sums
                E = ebp.tile([P, NV], F32R, tag="E")
                s1p = smp.tile([NL, 2], F32, tag="s1p")
                for nv in range(2):
                    nc.scalar.activation(
                        E[:NL, nv * 512 : (nv + 1) * 512],
                        psa[:, nv * 512 : (nv + 1) * 512],
                        AF.Exp, scale=ISQD, accum_out=s1p[:, nv, None],
                    )
                s1 = smp.tile([NL, 1], F32, tag="s1")
                nc.vector.reduce_sum(s1, s1p, axis=mybir.AxisListType.X)
                ivs1 = smp.tile([NL, 1], F32, tag="ivs1")
                nc.vector.reciprocal(ivs1, s1)
                return (E, ivs1)

            _emit_load_tg0(0)
            for b in range(BL):
                ls = b * NL  # column offset of this batch in *_all tensors

                FVn, FVTh, pre = vstate.pop(b)
                if pre is None:
                    pre = _emit_araw_E(b, FVn, FVTh)
                E, ivs1 = pre

                FMT = FMTs[b % 2]
                for nv in range(2):
                    ps2 = tp.tile([1, 512], F32, tag="tp")
                    nc.tensor.matmul(
                        ps2, onesc[:NL, :], E[:NL, nv * 512 : (nv + 1) * 512],
                        start=True, stop=True,
                    )
                    nc.vector.tensor_copy(FMT[96:97, nv * 512 : (nv + 1) * 512], ps2)


                # E^T blocks + s2 (column sums of E)
                ET = smp.tile([P, 8, NL], F32R, tag="ET")
                s2 = smp.tile([P, 8], F32, tag="s2")
                for tg in range(2):
                    ps = tp.tile([P, 512], F32R, tag="tp")
                    for tt in range(4):
                        t = tg * 4 + tt
                        nc.tensor.transpose(
                            ps[:, tt * P : (tt + 1) * P],
                            E[:, t * P : (t + 1) * P],
                            iden,
                        )
                    psv = ps.rearrange("p (four c) -> p four c", four=4)[:, :, :NL]
                    nc.scalar.activation(ET[:, tg * 4 : (tg + 1) * 4, :], psv, AF.Copy)
                    nc.vector.reduce_sum(
                        s2[:, tg * 4 : (tg + 1) * 4],
                        ET[:, tg * 4 : (tg + 1) * 4, :],
                        axis=mybir.AxisListType.X,
                    )
                ivs2 = smp.tile([P, 8], F32, tag="ivs2")
                nc.vector.reciprocal(ivs2, s2)

                if pending_finals:
                    pending_finals.pop(0)()

                # h1 = E @ fv -> [77, 768]; scaled by 1/s1 on copy-back
                psh = accp.tile([NL, DV], F32, tag="acc")
                for c0, cw in ((0, 512), (512, 256)):
                    sl = psh[:, c0 : c0 + cw]
                    for t in range(8):
                        nc.tensor.matmul(
                            sl, ET[:, t, :], FVn[:, t, c0 : c0 + cw],
                            start=(t == 0), stop=(t == 7),
                        )
                h1n = smp.tile([P, DV], F32R, tag="h1n")
                nc.scalar.activation(h1n[:NL, :], psh, AF.Identity, scale=ivs1)

                # H1T = h1n^T : [768, 77]
                H1T = smp.tile([P, 6, NL + 1], F32R, tag="H1T")
                for kg in range(2):
                    ps = tp.tile([P, 384], F32R, tag="tp")
                    for kk in range(3):
                        ko = kg * 3 + kk
                        nc.tensor.transpose(
                            ps[:, kk * P : (kk + 1) * P],
                            h1n[:, ko * P : (ko + 1) * P],
                            iden,
                        )
                    psv = ps.rearrange("p (three c) -> p three c", three=3)[:, :, : NL + 1]
                    if kg == 0:
                        nc.vector.tensor_copy(H1T[:, kg * 3 : (kg + 1) * 3, :], psv)
                    else:
                        nc.scalar.activation(H1T[:, kg * 3 : (kg + 1) * 3, :], psv, AF.Copy)

                if b + 1 < BL:
                    _emit_load_tg0(b + 1)

                if pending_finals:
                    pending_finals.pop(0)()

                # m_small = (fv_l @ W_vv^T) @ h1n^T + (fv_l @ b_vv) 1^T : [77, 78]
                MS = smp.tile([NL, NL + 1], F32R, tag="MS")
                psm = tp.tile([NL, NL + 1], F32, tag="tp")
                for ko in range(6):
                    nc.tensor.matmul(
                        psm, FWVT[:, ko, ls : ls + NL], H1T[:, ko, :],
                        start=(ko == 0), stop=False,
                    )
                nc.tensor.matmul(
                    psm, Cv[:1, ls : ls + NL], onesr[:1, : NL + 1],
                    start=False, stop=True,
                )
                nc.vector.tensor_copy(MS, psm)

                if pending_finals:
                    pending_finals.pop(0)()

                if b + 1 in vstate and len(vstate[b + 1][1]) == 1:
                    vstate[b + 1][1].append(_emit_tg(vstate[b + 1][0], 1))

                # fmT_un = m_small^T @ E : [77, 1024]; row 77 <- s2 row
                psf = accp.tile([NL, NV], F32, tag="acc")
                for nv in range(2):
                    nc.tensor.matmul(
                        psf[:, nv * 512 : (nv + 1) * 512],
                        MS[:, :NL], E[:NL, nv * 512 : (nv + 1) * 512],
                        start=True, stop=True,
                    )
                nc.vector.tensor_copy(FMT[:NL, :512], psf[:, :512])
                nc.scalar.activation(FMT[:NL, 512:], psf[:, 512:], AF.Copy)

                # finals for this batch are emitted during the NEXT batch
                # (software pipelining: their PE/copy/DMA work fills the
                # next batch's dependency stalls)
                def _emit_finals(b=b, FMT=FMT, ivs2=ivs2, ts=None):
                    for t in (ts if ts is not None else range(8)):
                        pso = accp.tile([P, OD], F32, tag="acc")
                        for c0, cw in ((0, 512), (512, 256)):
                            nc.tensor.matmul(
                                pso[:, c0 : c0 + cw],
                                FMT[:, t * P : (t + 1) * P],
                                Wm[:, c0 : c0 + cw],
                                start=True, stop=True,
                            )
                        OT = outp.tile([P, OD], F32, tag="OT")
                        if t % 2 == 0:
                            nc.vector.tensor_scalar_mul(OT, pso, ivs2[:, t, None])
                        else:
                            nc.scalar.activation(
                                OT, pso, AF.Identity, scale=ivs2[:, t, None]
                            )
                        nc.sync.dma_start(out_d[b, t * P : (t + 1) * P, :], OT)
                import functools as _ft
                pending_finals.append(_ft.partial(_emit_finals, ts=range(0, 3)))
                pending_finals.append(_ft.partial(_emit_finals, ts=range(3, 6)))
                pending_finals.append(_ft.partial(_emit_finals, ts=range(6, 8)))

                if b + 1 in vstate:
                    vstate[b + 1][2] = _emit_araw_E(
                        b + 1, vstate[b + 1][0], vstate[b + 1][1]
                    )

            for f in pending_finals:
                f()
            fvn3_pool.__exit__(None, None, None)

    nc.compile()
    return nc


_NC_CACHE = None
_last_in_maps = None


def _get_nc():
    global _NC_CACHE
    if _NC_CACHE is None:
        _NC_CACHE = _build()
    return _NC_CACHE


def kernel(**inputs) -> np.ndarray:
    fv = inputs["fv"]
    fl = inputs["fl"]
    consts = {
        "wkc": round_f32r(np.asarray(inputs["W_lk"]) @ np.asarray(inputs["W_vk"]).T),
        "wvc": round_f32r(np.asarray(inputs["W_lv"]) @ np.asarray(inputs["W_vv"]).T),
        "c2v": np.ascontiguousarray(
            np.asarray(inputs["W_vv"]) @ np.asarray(inputs["b_lv"]), dtype=np.float32
        ),
        "wcv": round_f32r(np.asarray(inputs["W_lv"]) @ np.asarray(inputs["b_vv"])),
        "ccv": np.array(
            [float(np.asarray(inputs["b_lv"]) @ np.asarray(inputs["b_vv"]))],
            dtype=np.float32,
        ),
        "wm": round_f32r(
            np.concatenate(
                [
                    np.asarray(inputs["W_m"]),
                    np.zeros((19, OD), np.float32),
                    np.asarray(inputs["b_m"])[None, :],
                ],
                axis=0,
            )
        ),
        "wc": round_f32r(np.asarray(inputs["W_lk"]) @ np.asarray(inputs["b_vk"])),
        "cc": np.array(
            [float(np.asarray(inputs["b_lk"]) @ np.asarray(inputs["b_vk"]))],
            dtype=np.float32,
        ),
        "c2": np.ascontiguousarray(
            np.asarray(inputs["W_vk"]) @ np.asarray(inputs["b_lk"]), dtype=np.float32
        ),
        "iden": np.eye(P, dtype=np.float32),
        "onesr": np.ones((1, 512), dtype=np.float32),
        "onesc": np.ones((P, 1), dtype=np.float32),
        "zeros": np.zeros((19, NV), dtype=np.float32),
    }
    fvr = round_f32r(fv)
    flr = round_f32r(fl)
    in_maps = []
    for c in range(NCORES):
        m = dict(consts)
        m["fv"] = np.ascontiguousarray(fvr[c * BL : (c + 1) * BL])
        m["fl"] = np.ascontiguousarray(flr[c * BL : (c + 1) * BL])
        in_maps.append(m)

    global _last_in_maps
    _last_in_maps = in_maps
    nc = _get_nc()
    res = run_bass_kernel_spmd(nc, in_maps, core_ids=list(range(NCORES)))
    out = np.concatenate([res.results[c]["out"] for c in range(NCORES)], axis=0)
    return np.ascontiguousarray(out, dtype=np.float32)



# revision 10
# speedup vs baseline: 1.4789x; 1.4789x over previous
"""Trainium2 Bass kernel for DenseLanguageGuidanceModule.

Math (per batch b), with the entire language-side computed EXACTLY on host
(fl is tiny: [B, 77, 512]):
    host:  g    = (fl@W_lk + b_lk) @ W_vk^T * isqd      [B, 77, 768]
           cst  = (fl@W_lk + b_lk) @ b_vk * isqd        [B, 77]
           fwv  = (fl@W_lv + b_lv) @ W_vv^T             [B, 77, 768]
           cv   = (fl@W_lv + b_lv) @ b_vv               [B, 77]
           wm   = [W_m; b_m; sum_l W_m[l]]              [79, 768]
    device:
           E    = exp(g @ fv^T + cst 1^T)               [77, 1024]
           h1T  = (E @ fv)^T                            [768, 77]   (raw)
           psm  = fwv @ h1T-chunks                      [77, 77]    (raw)
           MST  = [psm | 1 | cv]                        [77, 79]
           psf  = MST^T @ E                             [79, 1024]
                  rows 0..76 = m^T E (raw), row 77 = colsum(E), row 78 = cv^T E
           FMT  = diag([1/s1; 1; 1]) psf                (s1 = rowsum E)
           out[vt] = diag(1/s2) FMT[:, vt]^T @ wm       (s2 = colsum E)
    The wm extra rows make out = fm @ W_m + b_m exact:  row 77 pairs
    colsum(E)*b_m (1/s2-scaled -> +b_m), row 78 pairs cv^T E * sum(W_m)
    (the cv column of m_small is constant across l', so it multiplies
    sum_l' W_m[l']).

Distribution: pure data-parallel over batch B=32 across 8 NeuronCores
(4 batches per core), no collectives. All device tensors fp16 (bf16 where
range demands: E, h1T, MST, FMT); PSUM accumulation fp32.
"""
import sys

sys.path.insert(0, "/opt/trn_rl_repo")

import ml_dtypes
import numpy as np

import concourse.bass as bass  # noqa: E402
import concourse.tile as tile  # noqa: E402
from concourse import bacc, mybir  # noqa: E402
from concourse.bass_utils import run_bass_kernel_spmd  # noqa: E402

P = 128
NCORES = 8
B = 32
BL = B // NCORES          # 4 batches per core
NV, DV = 1024, 768        # vision tokens / dim
NL, DL = 77, 512          # language tokens / dim
D = 512                   # shared feature dim
OD = 768                  # output dim
KT = NL + 2               # 79: psf/finals contraction (m rows + ones + cv)

F32 = mybir.dt.float32
F16 = mybir.dt.float16
BF16 = mybir.dt.bfloat16
ISQD = 1.0 / float(np.sqrt(np.float32(D)))

AF = mybir.ActivationFunctionType
AX = mybir.AxisListType


def _build():
    nc = bacc.Bacc("TRN2", target_bir_lowering=False)

    fv_d = nc.dram_tensor("fv", [BL, NV, DV], F16, kind="ExternalInput")
    gt_d = nc.dram_tensor("gt", [DV, BL * NL], F16, kind="ExternalInput")
    fwvt_d = nc.dram_tensor("fwvt", [DV, BL * NL], F16, kind="ExternalInput")
    wm_d = nc.dram_tensor("wm", [KT, OD], F16, kind="ExternalInput")
    cc_d = nc.dram_tensor("cc", [NL, 2 * BL], F32, kind="ExternalInput")
    iden_d = nc.dram_tensor("iden", [P, P], F16, kind="ExternalInput")
    idenb_d = nc.dram_tensor("idenb", [P, P], BF16, kind="ExternalInput")
    out_d = nc.dram_tensor("out", [BL, NV, OD], F16, kind="ExternalOutput")

    with tile.TileContext(nc) as tc:
        with (
            tc.tile_pool(name="consts", bufs=1) as cp,
            tc.tile_pool(name="fvn", bufs=2) as fvnp,
            tc.tile_pool(name="fvt", bufs=2) as fvtp,
            tc.tile_pool(name="eb", bufs=2) as ebp,
            tc.tile_pool(name="sm", bufs=2) as smp,
            tc.tile_pool(name="outp", bufs=4) as outp,
            tc.tile_pool(name="tp", bufs=4, space="PSUM") as tp,     # 1-bank
            tc.tile_pool(name="acc", bufs=2, space="PSUM") as accp,  # 2-bank
        ):
            # ---- constants ----
            iden = cp.tile([P, P], F16)
            nc.sync.dma_start(iden, iden_d[:, :])
            idenb = cp.tile([P, P], BF16)
            nc.sync.dma_start(idenb, idenb_d[:, :])
            GT = cp.tile([P, 6, BL * NL], F16)
            FWVT = cp.tile([P, 6, BL * NL], F16)
            CC = cp.tile([NL, 2 * BL], F32)
            Wm = cp.tile([KT, OD], F16)
            FMTs = []
            for _i in range(2):
                _f = cp.tile([KT, NV], BF16, tag=f"FMT{_i}")
                FMTs.append(_f)

            pending = []  # deferred finals closures (one per t-pair)
            vstate = {}

            def emit_load(b):
                FVn = fvnp.tile([P, 8, DV], F16, tag="fvn")
                fvb = fv_d[b].rearrange("(t p) d -> p t d", p=P)
                for q in range(4):
                    nc.sync.dma_start(
                        FVn[:, 2 * q : 2 * q + 2, :], fvb[:, 2 * q : 2 * q + 2, :]
                    )
                vstate[b] = FVn

            # copy-engine rotation helper
            def ecopy(eng, dst, src, scale=None):
                if eng == "v":
                    if scale is None:
                        nc.vector.tensor_copy(dst, src)
                    else:
                        nc.vector.tensor_scalar_mul(dst, src, scale)
                elif eng == "a":
                    if scale is None:
                        nc.scalar.activation(dst, src, AF.Copy)
                    else:
                        nc.scalar.activation(dst, src, AF.Identity, scale=scale)
                else:
                    if scale is None:
                        nc.gpsimd.tensor_copy(dst, src)
                    else:
                        nc.gpsimd.tensor_scalar_mul(dst, src, scale)

            emit_load(0)
            nc.sync.dma_start(GT, gt_d.rearrange("(ko p) n -> p ko n", p=P))
            nc.sync.dma_start(CC, cc_d[:, :])
            nc.sync.dma_start(FWVT, fwvt_d.rearrange("(ko p) n -> p ko n", p=P))
            nc.sync.dma_start(Wm, wm_d[:, :])

            TENG = ["v", "a", "v", "a", "v", "a", "v", "a"]
            FENG = ["v", "a", "v", "a", "v", "a", "v", "a"]

            for b in range(BL):
                FVn = vstate.pop(b)
                # ---- fv^T: t-major transpose groups ----
                FVT = fvtp.tile([P, 6, NV], F16, tag="fvt")
                for t in range(8):
                    ps = tp.tile([P, 6 * P], F16, tag="tp")
                    for ko in range(6):
                        nc.tensor.transpose(
                            ps[:, ko * P : (ko + 1) * P],
                            FVn[:, t, ko * P : (ko + 1) * P],
                            iden,
                        )
                    psv = ps.rearrange("p (ko c) -> p ko c", ko=6)
                    ecopy(TENG[t], FVT[:, :, t * P : (t + 1) * P], psv)
                    if t % 2 == 1 and pending:
                        pending.pop(0)()

                if b + 1 < BL:
                    emit_load(b + 1)

                # ---- a_raw = g @ fv^T (+cst via exp bias) ----
                psa = accp.tile([NL, NV], F32, tag="acc")
                for nv in range(2):
                    sl = psa[:, nv * 512 : (nv + 1) * 512]
                    for ko in range(6):
                        nc.tensor.matmul(
                            sl,
                            GT[:, ko, b * NL : (b + 1) * NL],
                            FVT[:, ko, nv * 512 : (nv + 1) * 512],
                            start=(ko == 0),
                            stop=(ko == 5),
                        )
                if pending:
                    pending.pop(0)()

                # ---- E = exp(a + cst), s1 accumulated ----
                E = ebp.tile([P, NV], BF16, tag="E")
                s1p = smp.tile([NL, 1], F32, tag="s1p")
                nc.scalar.activation(
                    E[:NL, :], psa, AF.Exp, bias=CC[:, b, None], accum_out=s1p
                )
                ivs1x = smp.tile([KT, 1], F32, tag="ivs1x")
                nc.vector.memset(ivs1x, 1.0)
                nc.vector.reciprocal(ivs1x[:NL, :], s1p)

                # ---- E^T + column sums ----
                ET = smp.tile([P, 8, NL], BF16, tag="ET")
                for tg in range(2):
                    ps = tp.tile([P, 4 * P], BF16, tag="tp")
                    for tt in range(4):
                        t = tg * 4 + tt
                        nc.tensor.transpose(
                            ps[:, tt * P : (tt + 1) * P],
                            E[:, t * P : (t + 1) * P],
                            idenb,
                        )
                    psv = ps.rearrange("p (four c) -> p four c", four=4)[:, :, :NL]
                    ecopy("v" if tg == 0 else "a", ET[:, tg * 4 : (tg + 1) * 4, :], psv)
                s2 = smp.tile([P, 8], F32, tag="s2")
                nc.vector.reduce_sum(s2, ET, axis=AX.X)
                ivs2 = smp.tile([P, 8], F32, tag="ivs2")
                nc.vector.reciprocal(ivs2, s2)

                if pending:
                    pending.pop(0)()

                # ---- h1T = (E @ fv)^T, raw ----
                psh = tp.tile([P, 6, NL], F32, tag="tp")
                for m in range(6):
                    for t in range(8):
                        nc.tensor.matmul(
                            psh[:, m, :],
                            FVn[:, t, m * P : (m + 1) * P],
                            ET[:, t, :],
                            start=(t == 0),
                            stop=(t == 7),
                        )
                H1T = smp.tile([P, 6, NL], BF16, tag="H1T")
                nc.scalar.activation(H1T, psh, AF.Copy)

                if pending:
                    pending.pop(0)()

                # ---- psm = fwv @ h1T : [77, 77] raw ----
                psm = tp.tile([NL, NL], F32, tag="tp")
                for ko in range(6):
                    nc.tensor.matmul(
                        psm,
                        FWVT[:, ko, b * NL : (b + 1) * NL],
                        H1T[:, ko, :],
                        start=(ko == 0),
                        stop=(ko == 5),
                    )
                MST = smp.tile([NL, KT], BF16, tag="MST")
                nc.vector.tensor_copy(MST[:, :NL], psm)
                nc.vector.memset(MST[:, NL : NL + 1], 1.0)
                nc.vector.tensor_copy(MST[:, NL + 1 : KT], CC[:, BL + b, None])

                # ---- psf = MST^T @ E -> FMT (scaled by [1/s1; 1; 1]) ----
                psf = accp.tile([KT, NV], F32, tag="acc")
                for nv in range(2):
                    nc.tensor.matmul(
                        psf[:, nv * 512 : (nv + 1) * 512],
                        MST,
                        E[:NL, nv * 512 : (nv + 1) * 512],
                        start=True,
                        stop=True,
                    )
                FMT = FMTs[b % 2]
                nc.vector.tensor_scalar_mul(FMT[:, :512], psf[:, :512], ivs1x)
                nc.scalar.activation(
                    FMT[:, 512:], psf[:, 512:], AF.Identity, scale=ivs1x
                )

                # ---- finals: deferred to next batch's stream ----
                def emit_final_pair(b=b, FMT=FMT, ivs2=ivs2, tp_=0):
                    OT2 = outp.tile([P, 2, OD], F16, tag="OT2")
                    for i in range(2):
                        t = 2 * tp_ + i
                        ps5 = tp.tile([P, 512], F32, tag="tp")
                        nc.tensor.matmul(
                            ps5, FMT[:, t * P : (t + 1) * P], Wm[:, :512],
                            start=True, stop=True,
                        )
                        ps2 = tp.tile([P, 256], F32, tag="tp")
                        nc.tensor.matmul(
                            ps2, FMT[:, t * P : (t + 1) * P], Wm[:, 512:],
                            start=True, stop=True,
                        )
                        ecopy(FENG[2 * t % 8], OT2[:, i, :512], ps5,
                              scale=ivs2[:, t, None])
                        ecopy(FENG[(2 * t + 1) % 8], OT2[:, i, 512:], ps2,
                              scale=ivs2[:, t, None])
                    nc.sync.dma_start(
                        out_d[b, tp_ * 256 : (tp_ + 1) * 256, :].rearrange(
                            "(t p) d -> p t d", p=P
                        ),
                        OT2,
                    )

                import functools as _ft

                for tp_ in range(4):
                    pending.append(_ft.partial(emit_final_pair, tp_=tp_))

            for f in pending:
                f()

    nc.compile()
    return nc


_NC_CACHE = None
_last_in_maps = None


def _get_nc():
    global _NC_CACHE
    if _NC_CACHE is None:
        _NC_CACHE = _build()
    return _NC_CACHE


def kernel(**inputs) -> np.ndarray:
    f32 = np.float32
    fv = np.asarray(inputs["fv"], f32)
    fl = np.asarray(inputs["fl"], f32)
    W_vk = np.asarray(inputs["W_vk"], f32)
    b_vk = np.asarray(inputs["b_vk"], f32)
    W_vv = np.asarray(inputs["W_vv"], f32)
    b_vv = np.asarray(inputs["b_vv"], f32)
    W_lk = np.asarray(inputs["W_lk"], f32)
    b_lk = np.asarray(inputs["b_lk"], f32)
    W_lv = np.asarray(inputs["W_lv"], f32)
    b_lv = np.asarray(inputs["b_lv"], f32)
    W_m = np.asarray(inputs["W_m"], f32)
    b_m = np.asarray(inputs["b_m"], f32)

    # exact language-side precompute (host)
    fkl = fl @ W_lk + b_lk                    # [B, 77, 512]
    g = (fkl @ W_vk.T) * ISQD                 # [B, 77, 768]
    cst = (fkl @ b_vk) * ISQD                 # [B, 77]
    fvl = fl @ W_lv + b_lv
    fwv = fvl @ W_vv.T                        # [B, 77, 768]
    cv = fvl @ b_vv                           # [B, 77]
    wm = np.concatenate(
        [W_m, b_m[None, :], W_m.sum(axis=0)[None, :]], axis=0
    )                                          # [79, 768]

    consts = {
        "wm": wm.astype(np.float16),
        "iden": np.eye(P, dtype=np.float16),
        "idenb": np.eye(P, dtype=np.float32).astype(ml_dtypes.bfloat16),
    }
    in_maps = []
    for c in range(NCORES):
        cb = slice(c * BL, (c + 1) * BL)
        m = dict(consts)
        m["fv"] = np.ascontiguousarray(fv[cb].astype(np.float16))
        m["gt"] = np.ascontiguousarray(
            g[cb].transpose(2, 0, 1).reshape(DV, BL * NL).astype(np.float16)
        )
        m["fwvt"] = np.ascontiguousarray(
            fwv[cb].transpose(2, 0, 1).reshape(DV, BL * NL).astype(np.float16)
        )
        m["cc"] = np.ascontiguousarray(
            np.concatenate([cst[cb].T, cv[cb].T], axis=1).astype(f32)
        )                                      # [77, 2*BL]
        in_maps.append(m)

    global _last_in_maps
    _last_in_maps = in_maps
    nc = _get_nc()
    res = run_bass_kernel_spmd(nc, in_maps, core_ids=list(range(NCORES)))
    out = np.concatenate([res.results[c]["out"] for c in range(NCORES)], axis=0)
    return np.ascontiguousarray(out.astype(np.float32))
